# revision 23
# baseline (speedup 1.0000x reference)
"""Trainium2 Bass kernel for nn_CB_RNN_tiedcell (H=24, IN=8, B=1048576).

Math
----
reference(x, W, P, ...) computes, per batch column b:
    z_t = dt*sig(K@r + P_z@x_b + b_z)      (K, P_z, r, biases batch-constant)
    v   = (1-z_t)*v0 + dt*(W@(U*X*r) + P@x_b + b_v)
All (24,1) state math is batch-constant and precomputed on the host.  When
v0 == 0 (the shipped inputs) the sigmoid path vanishes and
    v[:,b] = dt*P@x_b + cv.

Kernel design (v2: weight-stationary, uint8 output)
---------------------------------------------------
* Stationary [42, 120]: 5 block-diagonal copies of (dt*P)^T (rows c*8+k,
  cols c*24+h) + two bias rows (fp16 cv hi/lo split).  Loaded into the PE
  array once; x streams through as the moving operand.
* Moving x: host prepacks shard into [42, 52*512] fp16 where partition
  c*8+k holds x[k, base_t + c*512 + w] for tile t (2560 batches each;
  tile 51 overlaps 50 to cover B_c=131072).  Rows 40/41 are constant 1.0
  (bias).  Every DMA span is fully contiguous per partition.
* One matmul per tile: out PSUM [120, 512] (one bank); groups of 4 tiles
  share a 4-bank PSUM tile; psum already equals v (bias in-matmul).
* Cast pass v -> uint8 with per-partition scale 1/step_h, +128.5 offset,
  split round-robin across Act / Pool / DVE so no single engine bottlenecks.
  Host dequantizes (u8 - 128) * step_h; step_h from a per-row bound on |v|.
* All DMA triggers (x in, stores out) on the SP(sync) HWDGE queue.
* I/O per core: 2 MiB fp16 in + 3.05 MiB u8 out (was 2 + 6 fp16).
"""

import numpy as np

H = 24
IN = 8
NCORES = 8
B_FULL = 1048576

S = 5                    # chunks packed per matmul (5*24 = 120 out rows)
KROWS = S * IN + 2       # 42 contraction rows (40 x rows + 2 bias rows)
MROWS = S * H            # 120
TILE_B = S * 512         # 2560 batches per tile
NT = 52                  # tiles per core (51 + 1 overlapping tail)
GROUP = 2                # tiles per PSUM tile (2 banks; 4 bufs = all 8 banks)
STORE_G = 4              # per-engine groups per output store
QS_TILES = [1, 2, 4, 8, 12, 12, 13]   # x supertile sizes in tiles (sum 52)
U8_OFF = 128.0           # float->u8 rounds to nearest (probed on DVE/Act/Pool)

# fp8 DoubleRow mode: contraction split into hi/lo terms so e4m3's 6% grid
# cancels: W_hi@x_hi + W_hi@x_lo + W_lo@x_hi + bias = 122 logical rows,
# packed as 2 k-tiles of K8=61 partitions for perf_mode=DoubleRow (PE
# processes 2 output columns/cycle for fp8).
FP8_MODE = False
K8 = 61
SC8 = np.float32(64.0)   # weight prescale so dt*P fits e4m3 normals


def _np_softplus(x):
    x = np.asarray(x, np.float32)
    return np.logaddexp(np.float32(0.0), x).astype(np.float32)


def _np_sigmoid(x):
    x = np.asarray(x, np.float32)
    return (np.float32(1.0) / (np.float32(1.0) + np.exp(-x))).astype(np.float32)


def host_precompute(W, P, b_v, b_z, e, e_p, c_x, c_u, c_U, v0, X0, U0):
    """All (24,1)/(24,24) batch-constant math, in float32 mirroring the ref."""
    dt = np.float32(0.1)
    delta_t = np.float32(1.0)
    z_min, z_max = np.float32(0.001), np.float32(0.1)
    sp, sig = _np_softplus, _np_sigmoid

    W = np.asarray(W, np.float32)
    P = np.asarray(P, np.float32)
    b_v = np.asarray(b_v, np.float32).reshape(H, 1)
    b_z = np.asarray(b_z, np.float32).reshape(H, 1)
    v0 = np.asarray(v0, np.float32).reshape(H, 1)
    X0 = np.asarray(X0, np.float32).reshape(H, 1)
    U0 = np.asarray(U0, np.float32).reshape(H, 1)
    c_x = np.asarray(c_x, np.float32).reshape(H, 1)
    c_u = np.asarray(c_u, np.float32).reshape(H, 1)
    c_U = np.asarray(c_U, np.float32).reshape(H, 1)

    K = sp(np.float32(e).reshape(())) * sp(W)        # (H,H)
    P_z = sp(np.float32(e_p).reshape(())) * sp(P)    # (H,IN)

    r = sig(v0)                                      # (H,1)
    z_x = z_min + (z_max - z_min) * sig(c_x)
    X = z_x + (np.float32(1.0) - z_x) * X0 - delta_t * U0 * X0 * r
    z_u = z_min + (z_max - z_min) * sig(c_u)
    Ucap = np.float32(0.9) * sig(c_U)
    U = Ucap * z_u + (np.float32(1.0) - z_u) * U0 + delta_t * Ucap * (np.float32(1.0) - U0) * r
    U_c = np.clip(U, Ucap, np.float32(1.0))          # (H,1), batch-constant

    zpre = (K @ r + b_z).astype(np.float32)          # (H,1)
    u_vec = (U_c * X * r).astype(np.float32)         # (H,1)
    bias_v = (W @ u_vec + b_v).astype(np.float32)    # (H,1)

    w_v = (dt * P).T.astype(np.float32).copy()       # (IN,H)
    cv = (dt * bias_v + (np.float32(1.0) - dt) * v0).reshape(H).astype(np.float32)
    w_z = (-P_z).T.astype(np.float32).copy()         # (IN,H)
    cz = (-zpre).reshape(H).astype(np.float32)
    dtv0 = (dt * v0).reshape(H).astype(np.float32)
    return w_v, cv, w_z, cz, dtv0


# ---------------------------------------------------------------------------
# v2 fast path (v0 == 0): weight-stationary matmul, uint8 output
# ---------------------------------------------------------------------------

def _tile_bases(B_c):
    """Start batch of each of the NT tiles; the last tile overlaps so NT*TILE_B
    covers B_c exactly once plus a duplicated span the host ignores."""
    bases = [t * TILE_B for t in range(NT - 1)]
    bases.append(B_c - TILE_B)
    return bases


def pack_weights(w_v, cv):
    """Stationary [KROWS, 128] fp16: block-diag (dt*P)^T + cv hi/lo rows.
    Columns 120..127 are zero padding so NumWeights==128 triggers the PE's
    fast weight load path."""
    Wfull = np.zeros((KROWS, 128), np.float32)
    for c in range(S):
        Wfull[c * IN : (c + 1) * IN, c * H : (c + 1) * H] = w_v  # (IN,H) block
    cv_hi = cv.astype(np.float16).astype(np.float32)
    cv_lo = (cv - cv_hi).astype(np.float32)
    for c in range(S):
        Wfull[S * IN, c * H : (c + 1) * H] = cv_hi
        Wfull[S * IN + 1, c * H : (c + 1) * H] = cv_lo
    return Wfull.astype(np.float16)


def pack_x_shard(x_shard_f16, bases):
    """[42, NT*512] fp16: partition c*8+k <- x[k, base_t + c*512 + w]."""
    out = np.empty((KROWS, NT * 512), np.float16)
    xs = x_shard_f16                                     # (IN, B_c)
    for t, base in enumerate(bases):
        blk = xs[:, base : base + TILE_B].reshape(IN, S, 512)  # k, c, w
        out[:S * IN, t * 512 : (t + 1) * 512] = (
            blk.transpose(1, 0, 2).reshape(S * IN, 512))
    out[S * IN :, :] = np.float16(1.0)
    return out


def make_scales(w_v, cv, maxx):
    """Per-row bound b_h on |v| -> step_h, svec = 1/step_h (f32)."""
    b = (np.abs(w_v.T) @ maxx.reshape(IN, 1)).reshape(H) + np.abs(cv)  # (H,)
    step = (b * np.float32(1.02) / np.float32(127.0)).astype(np.float32)
    svec = (np.float32(1.0) / step).astype(np.float32)
    sv_col = np.tile(svec, S).reshape(MROWS, 1)       # per-partition scale
    step_col = np.tile(step, S).reshape(MROWS, 1)
    return sv_col, step_col


def _f8(a):
    import ml_dtypes
    return np.asarray(a, np.float32).astype(ml_dtypes.float8_e4m3)


def _rows122(w_v, cv):
    """Logical [122, 128] f32 stationary (pre-fp8): 3 product terms + bias."""
    blk_hi = np.zeros((40, 128), np.float32)
    blk_lo = np.zeros((40, 128), np.float32)
    w_hi = _f8(w_v * SC8).astype(np.float32)        # (IN,H)
    w_lo = (w_v * SC8 - w_hi).astype(np.float32)
    for c in range(S):
        blk_hi[c * IN : (c + 1) * IN, c * H : (c + 1) * H] = w_hi
        blk_lo[c * IN : (c + 1) * IN, c * H : (c + 1) * H] = w_lo
    cv_hi = _f8(cv * SC8).astype(np.float32)
    cv_lo = (cv * SC8 - cv_hi).astype(np.float32)
    rows = np.zeros((2 * K8, 128), np.float32)
    rows[0:40] = blk_hi          # * x_hi
    rows[40:80] = blk_hi         # * x_lo
    rows[80:120] = blk_lo        # * x_hi
    for c in range(S):
        rows[120, c * H : (c + 1) * H] = cv_hi
        rows[121, c * H : (c + 1) * H] = cv_lo
    return rows


def pack_weights_fp8(w_v, cv):
    """[K8, 2*128] fp8 stationary for DoubleRow ([q, i, m] with i-stride 128)."""
    rows = _rows122(w_v, cv)
    out = np.empty((K8, 2, 128), np.float32)
    for i in range(2):
        out[:, i, :] = rows[i * K8 : (i + 1) * K8]
    return _f8(out).reshape(K8, 256)


def pack_x_shard_fp8(x_shard_f32, bases):
    """[K8, NT*1024] fp8 moving: tile t cols [t*1024+i*512+w] hold logical
    row i*K8+q: x_hi rows 0..39, x_lo rows 40..79, x_hi again 80..119,
    ones 120/121."""
    import ml_dtypes
    f8 = ml_dtypes.float8_e4m3
    xs = np.asarray(x_shard_f32, np.float32)
    x_hi = xs.astype(f8)
    x_lo = (xs - x_hi.astype(np.float32)).astype(f8)
    out = np.empty((2 * K8, NT * 512), f8)
    for t, base in enumerate(bases):
        hi = x_hi[:, base : base + TILE_B].reshape(IN, S, 512)
        lo = x_lo[:, base : base + TILE_B].reshape(IN, S, 512)
        hi_t = hi.transpose(1, 0, 2).reshape(40, 512)
        lo_t = lo.transpose(1, 0, 2).reshape(40, 512)
        sl = out[:, t * 512 : (t + 1) * 512]
        sl[0:40] = hi_t
        sl[40:80] = lo_t
        sl[80:120] = hi_t
        sl[120:122] = np.float32(1.0)
    # interleave k-tiles: partition q gets rows q and K8+q in adjacent 512s
    res = np.empty((K8, NT, 2, 512), f8)
    src = out.reshape(2, K8, NT, 512)
    res[:, :, 0, :] = src[0].transpose(0, 1, 2)
    res[:, :, 1, :] = src[1].transpose(0, 1, 2)
    return res.reshape(K8, NT * 1024)


def _store_plan(n_groups):
    """Fixed store schedule: [(engine, [group indices], dram offset elems)].
    Even psum-groups are cast by Act, odd by DVE; each engine bundles
    STORE_G consecutive of its groups per store DMA."""
    per_eng = {
        "scalar": [g for g in range(n_groups) if g % 2 == 0],
        "vector": [g for g in range(n_groups) if g % 2 == 1],
    }
    chunks = []
    for eng, gs in per_eng.items():
        for i in range(0, len(gs), STORE_G):
            chunks.append((eng, gs[i : i + STORE_G]))
    # interleave by first group index for temporal locality
    chunks.sort(key=lambda c: c[1][0])
    plan = []
    off = 0
    for eng, gs in chunks:
        plan.append((eng, gs, off))
        off += MROWS * len(gs) * GROUP * 512
    return plan


def build_program_v2():
    import concourse.bass as bass  # noqa: F401 (engine types via nc)
    import concourse.bacc as bacc
    import concourse.tile as tile
    from concourse import mybir

    f32 = mybir.dt.float32
    f16 = mybir.dt.float16
    u8 = mybir.dt.uint8
    AT = mybir.AluOpType
    AF = mybir.ActivationFunctionType

    nc = bacc.Bacc()
    x_in = nc.declare_dram_parameter("xs", [KROWS, NT * 512], f16, isOutput=False)
    wblk_in = nc.declare_dram_parameter("wblk", [KROWS, 128], f16, isOutput=False)
    sv_in = nc.declare_dram_parameter("svec", [MROWS, 1], f32, isOutput=False)
    out_ext = nc.declare_dram_parameter("out", [MROWS * NT * 512], u8, isOutput=True)

    n_groups = NT // GROUP
    with tile.TileContext(nc) as tc:
        with (
            tc.tile_pool(name="singles", bufs=1) as singles,
            tc.tile_pool(name="stg", bufs=4) as stg,
            tc.tile_pool(name="ps", bufs=2, space="PSUM") as psp,
        ):
            # x supertile loads lead; they alternate between the two HWDGE
            # queues (SP and Act — Act is idle until its first cast) so the
            # trigger+transfer chains run in parallel.  svec is only needed
            # by the first cast, so its trigger goes last.  All loads are
            # issued up-front: their triggers have no waits, so later
            # store-trigger waits cannot delay them.
            xt_tiles = []
            xt0 = singles.tile([KROWS, QS_TILES[0] * 512], f16, tag="xt0")
            nc.sync.dma_start(out=xt0, in_=x_in[:, : QS_TILES[0] * 512])
            xt_tiles.append((xt0, 0, QS_TILES[0]))

            wblk_sb = singles.tile([KROWS, 128], f16)
            nc.scalar.dma_start(out=wblk_sb, in_=wblk_in[:, :])

            off = QS_TILES[0] * 512
            for si, q in enumerate(QS_TILES[1:], start=1):
                xt = singles.tile([KROWS, q * 512], f16, tag=f"xt{si}")
                eng = nc.scalar if si % 2 else nc.sync
                eng.dma_start(out=xt, in_=x_in[:, off : off + q * 512])
                xt_tiles.append((xt, off // 512, q))
                off += q * 512

            sv_sb = singles.tile([MROWS, 1], f32)
            nc.sync.dma_start(out=sv_sb, in_=sv_in[:, :])

            def tile_view(t):
                for xt, t0, q in xt_tiles:
                    if t0 <= t < t0 + q:
                        return xt[:, (t - t0) * 512 : (t - t0 + 1) * 512]
                raise AssertionError(t)

            # PSUM is only readable by DVE and Act; casts alternate between
            # them per group.  Each engine accumulates its groups into its
            # own stage tiles (a shared tile would serialize the pair via
            # Tile's same-tile write tracking) and ships its own stores,
            # placed in DRAM per _store_plan().
            st_by_eng = {}
            fill_by_eng = {"scalar": 0, "vector": 0}
            plan = _store_plan(n_groups)
            store_of_group = {}
            for si, (eng, groups, off) in enumerate(plan):
                for gi in groups:
                    store_of_group[gi] = (si, eng)
            for g in range(n_groups):
                pt = psp.tile([128, GROUP * 512], f32, tag="pt")
                for q in range(GROUP):
                    nc.tensor.matmul(
                        pt[:, q * 512 : (q + 1) * 512],
                        wblk_sb[:, :],
                        tile_view(g * GROUP + q),
                        start=True, stop=True,
                    )
                si, eng = store_of_group[g]
                _, groups, off = plan[si]
                if fill_by_eng[eng] == 0:
                    st_new = stg.tile(
                        [MROWS, STORE_G * GROUP * 512], u8, tag=f"st_{eng}")
                    st_by_eng[eng] = st_new
                st = st_by_eng[eng]
                base = fill_by_eng[eng] * GROUP * 512
                sl = st[:, base : base + GROUP * 512]
                pv = pt[0:MROWS, :]
                if eng == "scalar":
                    nc.scalar.activation(
                        out=sl, in_=pv, func=AF.Copy,
                        bias=U8_OFF, scale=sv_sb[:, 0:1])
                else:
                    nc.vector.tensor_scalar(
                        out=sl, in0=pv,
                        scalar1=sv_sb[:, 0:1], scalar2=U8_OFF,
                        op0=AT.mult, op1=AT.add)
                fill_by_eng[eng] += 1
                if fill_by_eng[eng] == len(groups):
                    w = len(groups) * GROUP * 512
                    dst = out_ext[off : off + MROWS * w].rearrange(
                        "(m f) -> m f", m=MROWS)
                    nc.sync.dma_start(out=dst, in_=st[:, :w])
                    fill_by_eng[eng] = 0
    nc.compile()
    return nc


def unshard_core_v2(dev_flat, step_col, B_c):
    """Invert device layout -> (B_c, H) float32, dequantizing uint8."""
    bases = _tile_bases(B_c)
    dev = np.empty((MROWS, NT * 512), np.uint8)
    dev_flat = np.asarray(dev_flat)
    n_groups = NT // GROUP
    for eng, groups, off in _store_plan(n_groups):
        w = len(groups) * GROUP * 512
        slab = dev_flat[off : off + MROWS * w].reshape(MROWS, w)
        for j, gi in enumerate(groups):
            dev[:, gi * GROUP * 512 : (gi + 1) * GROUP * 512] = (
                slab[:, j * GROUP * 512 : (j + 1) * GROUP * 512])
    vals = (dev.astype(np.float32) - np.float32(128.0)) * step_col  # [120, NT*512]
    T = vals.reshape(S, H, NT, 512).transpose(2, 0, 3, 1)  # [t, c, w, h]
    out_core = np.empty((B_c, H), np.float32)
    for t, base in enumerate(bases):
        out_core[base : base + TILE_B] = T[t].reshape(TILE_B, H)
    return out_core


# ---------------------------------------------------------------------------
# legacy general path (v0 != 0): x-stationary block-diag kernel (from v1)
# ---------------------------------------------------------------------------

def _block_diag(w, Sv):
    out = np.zeros((128, Sv * H), np.float32)
    for c in range(Sv):
        for k in range(IN):
            out[k * Sv + c, H * c : H * c + H] = w[k]
    return out


def _pad_vec(v, Sv, PAIR):
    out = np.zeros((1, PAIR * 512), np.float32)
    for q in range(PAIR):
        out[0, 512 * q : 512 * q + Sv * H] = np.tile(v, Sv)
    return out


def _qsched(total):
    if total < 16:
        return [total]
    if total < 48 or (total - 32) % 16:
        return [4, 12] + [16] * ((total - 16) // 16)
    return [4, 12] + [16] * ((total - 32) // 16) + [8, 4, 4]


def build_program_legacy(B_c, qsched=None):
    import concourse.bass as bass  # noqa: F401
    import concourse.bacc as bacc
    import concourse.tile as tile
    from concourse import mybir

    Sv = 16
    CHB = B_c // Sv
    qsched = qsched or _qsched(B_c // (Sv * 128))
    assert sum(128 * q for q in qsched) == CHB, (qsched, CHB)
    N = Sv * H
    G = 2
    f32 = mybir.dt.float32
    f16 = mybir.dt.float16

    nc = bacc.Bacc()
    x_in = nc.declare_dram_parameter("xs", [IN, B_c], f16, isOutput=False)
    wblk_in = nc.declare_dram_parameter("wblk", [128, N], f16, isOutput=False)
    cvec_in = nc.declare_dram_parameter("cvec", [1, G * 512], f32, isOutput=False)
    wblkz_in = nc.declare_dram_parameter("wblkz", [128, N], f16, isOutput=False)
    czvec_in = nc.declare_dram_parameter("czvec", [1, G * 512], f32, isOutput=False)
    dvvec_in = nc.declare_dram_parameter("dvvec", [1, G * 512], f32, isOutput=False)
    out_ext = nc.declare_dram_parameter("out", [B_c * H], f16, isOutput=True)

    AT = mybir.AluOpType
    with tile.TileContext(nc) as tc:
        with (
            tc.tile_pool(name="singles", bufs=1) as singles,
            tc.tile_pool(name="op", bufs=4) as op,
            tc.tile_pool(name="ps", bufs=2, space="PSUM") as psp,
            tc.tile_pool(name="sp", bufs=4) as sbp,
        ):
            wblk_sb = singles.tile([128, N], f16)
            nc.sync.dma_start(out=wblk_sb, in_=wblk_in[:, :])
            cv_rep = singles.tile([128, G * 512], f32)
            wblkz_sb = singles.tile([128, N], f16)
            nc.sync.dma_start(out=wblkz_sb, in_=wblkz_in[:, :])
            cz_rep = singles.tile([128, G * 512], f32)
            dv_rep = singles.tile([128, G * 512], f32)

            def gv(t, g):
                return t.rearrange("p (q b) -> p q b", q=G)[:, 0:g, 0:N]

            off = 0
            flat = 0
            for T, QT in enumerate(qsched):
                SLICE = 128 * QT
                xt = singles.tile([128, SLICE], f16, tag=f"xt{T}")
                srcx = x_in[:, :].rearrange(
                    "k (c w) -> k c w", c=Sv)[:, :, off : off + SLICE]
                nc.sync.dma_start(out=xt[:, :], in_=srcx)
                if T == 0:
                    nc.gpsimd.dma_start(
                        out=cv_rep, in_=cvec_in[:, :].to_broadcast([128, G * 512]))
                    nc.gpsimd.dma_start(
                        out=cz_rep,
                        in_=czvec_in[:, :].to_broadcast([128, G * 512]))
                    nc.gpsimd.dma_start(
                        out=dv_rep,
                        in_=dvvec_in[:, :].to_broadcast([128, G * 512]))

                plan = [16] * (QT // 16) if QT > 16 else [QT]
                jbase = 0
                for JFc in plan:
                    out_sb = op.tile([128, JFc * Sv * H], f16, tag="osb")
                    for j0 in range(0, JFc, G):
                        g = min(G, JFc - j0)
                        pt = psp.tile([128, G * 512], f32, tag="pt")
                        for q in range(g):
                            lhsT = xt.rearrange(
                                "p (m q) -> p m q", q=QT)[:, :, jbase + j0 + q]
                            nc.tensor.matmul(pt[:, 512 * q : 512 * q + N], lhsT,
                                             wblk_sb, start=True, stop=True)
                        p_v = gv(pt, g)
                        c_v = gv(cv_rep, g)
                        o_v = out_sb.rearrange(
                            "p (j b) -> p j b", b=Sv * H)[:, j0 : j0 + g, :]
                        ptz = psp.tile([128, G * 512], f32, tag="ptz")
                        for q in range(g):
                            lhsT = xt.rearrange(
                                "p (m q) -> p m q", q=QT)[:, :, jbase + j0 + q]
                            nc.tensor.matmul(ptz[:, 512 * q : 512 * q + N],
                                             lhsT, wblkz_sb,
                                             start=True, stop=True)
                        zb = sbp.tile([128, G * N], f32)
                        zb_v = zb.rearrange("p (q b) -> p q b", q=G)[:, 0:g, :]
                        nc.vector.scalar_tensor_tensor(
                            out=zb_v, in0=gv(ptz, g), scalar=1.0,
                            in1=gv(cz_rep, g), op0=AT.mult, op1=AT.add,
                        )
                        sg = sbp.tile([128, G * N], f32)
                        nc.scalar.activation(
                            out=sg, in_=zb,
                            func=mybir.ActivationFunctionType.Sigmoid,
                        )
                        sg_v = sg.rearrange("p (q b) -> p q b", q=G)[:, 0:g, :]
                        tt = sbp.tile([128, G * N], f32)
                        tt_v = tt.rearrange("p (q b) -> p q b", q=G)[:, 0:g, :]
                        nc.vector.tensor_tensor(
                            out=tt_v, in0=sg_v, in1=gv(dv_rep, g), op=AT.mult,
                        )
                        nc.vector.scalar_tensor_tensor(
                            out=tt_v, in0=tt_v, scalar=1.0, in1=c_v,
                            op0=AT.mult, op1=AT.add,
                        )
                        nc.vector.scalar_tensor_tensor(
                            out=o_v, in0=gv(pt, g), scalar=1.0, in1=tt_v,
                            op0=AT.mult, op1=AT.add,
                        )

                    sz = 128 * JFc * Sv * H
                    dst_o = out_ext[flat : flat + sz].rearrange(
                        "(m f) -> m f", m=128)
                    nc.scalar.dma_start(out=dst_o, in_=out_sb[:, :])
                    flat += sz
                    jbase += JFc
                off += SLICE
    nc.compile()
    return nc


def unshard_core_legacy(dev_flat, qsched, B_c):
    Sv = 16
    CHB = B_c // Sv
    out_core = np.empty((Sv, CHB, H), np.float32)
    flat = 0
    off = 0
    for QT in qsched:
        plan = [16] * (QT // 16) if QT > 16 else [QT]
        jbase = 0
        dst = out_core[:, off : off + 128 * QT, :]
        for JFc in plan:
            sz = 128 * JFc * Sv * H
            piece = np.asarray(dev_flat[flat : flat + sz]).reshape(
                128, JFc, Sv, H).astype(np.float32)
            idx = (np.arange(128)[:, None] * QT + jbase
                   + np.arange(JFc)[None, :]).ravel()
            dst[:, idx, :] = piece.transpose(2, 0, 1, 3).reshape(Sv, 128 * JFc, H)
            flat += sz
            jbase += JFc
        off += 128 * QT
    return out_core.reshape(B_c, H)


def _run(nc, in_maps, core_ids, trace=False):
    from concourse.bass_utils import run_bass_kernel_spmd
    return run_bass_kernel_spmd(nc, in_maps, core_ids, trace=trace)


def kernel(x, W, P, b_v, b_z, e, e_p, c_x, c_u, c_U, v0, X0, U0,
           _trace=False):
    x = np.ascontiguousarray(np.asarray(x, np.float32))
    assert x.shape == (IN, B_FULL), x.shape
    w_v, cv, w_z, cz, dtv0 = host_precompute(
        W, P, b_v, b_z, e, e_p, c_x, c_u, c_U, v0, X0, U0)
    full_path = bool(np.any(dtv0 != 0))
    B_c = B_FULL // NCORES
    core_ids = list(range(NCORES))

    if not full_path:
        nc = build_program_v2()
        maxx = np.abs(x).max(axis=1).astype(np.float32)
        sv_col, step_col = make_scales(w_v, cv, maxx)
        wblk = pack_weights(w_v, cv)
        bases = _tile_bases(B_c)
        x16 = x.astype(np.float16)
        base_map = {"wblk": wblk, "svec": sv_col}
        in_maps = []
        for c in core_ids:
            m = dict(base_map)
            m["xs"] = pack_x_shard(x16[:, c * B_c : (c + 1) * B_c], bases)
            in_maps.append(m)
        res = _run(nc, in_maps, core_ids, trace=_trace)
        out = np.concatenate(
            [unshard_core_v2(res.results[i]["out"], step_col, B_c)
             for i in range(NCORES)], axis=0)
    else:
        qsched = _qsched(B_c // (16 * 128))
        nc = build_program_legacy(B_c, qsched=qsched)
        wblk = _block_diag(w_v, 16).astype(np.float16)
        base_map = {
            "wblk": wblk, "cvec": _pad_vec(cv, 16, 2),
            "wblkz": _block_diag(w_z, 16).astype(np.float16),
            "czvec": _pad_vec(cz, 16, 2), "dvvec": _pad_vec(dtv0, 16, 2),
        }
        in_maps = []
        for c in core_ids:
            m = dict(base_map)
            m["xs"] = np.ascontiguousarray(
                x[:, c * B_c : (c + 1) * B_c]).astype(np.float16)
            in_maps.append(m)
        res = _run(nc, in_maps, core_ids, trace=_trace)
        out = np.concatenate(
            [unshard_core_legacy(res.results[i]["out"], qsched, B_c)
             for i in range(NCORES)], axis=0)

    if _trace:
        kernel.last_exec_time_ns = res.exec_time_ns
        kernel.last_results = res
    return out


# revision 34
# speedup vs baseline: 1.3280x; 1.3280x over previous
"""Trainium2 Bass kernel for nn_CB_RNN_tiedcell (H=24, IN=8, B=1048576).

Math
----
reference(x, W, P, ...) computes, per batch column b:
    z_t = dt*sig(K@r + P_z@x_b + b_z)      (K, P_z, r, biases batch-constant)
    v   = (1-z_t)*v0 + dt*(W@(U*X*r) + P@x_b + b_v)
All (24,1) state math (r, X, U, Ucap, clamp, K@r, W@u) is batch-constant and
precomputed on the host.  With s = sig(-(P_z@x_b + zpre)) = 1 - sig(+...):
    v[:,b] = dt*P@x_b + cv + dtv0 * s[:,b]
where cv = dt*(W@u + b_v) + (1-dt)*v0 and dtv0 = dt*v0.  When v0 == 0 (the
shipped inputs) the sigmoid path vanishes; the program is built without it
(full_path=False) and a general program is built when v0 != 0.

Kernel design (pure data parallel, 8 cores, B/8 = 131072 batches each)
----------------------------------------------------------------------
* Block-diagonal stationary trick: one fp16 matmul per 2048 batches.  The
  PE stationary is a [128, 128] tile of x holding 16 independent 8-row
  sub-chunks (chunk c of the shard on partitions {k*16+c}); the moving
  operand is a constant block-diagonal weight matrix [128, 16*24].  One
  matmul yields batch-major [128, 384] PSUM = 16 chunks x 128 batches.
* x is host-cast to fp16 and laid out so every per-partition DMA span is
  fully contiguous; supertile sizes ramp 4,12,16,...,8,4,4 so the first
  matmul and first store start early and the final ship-out tail is short.
* 4 matmuls share one 4-bank PSUM tile; a single fused DVE
  scalar_tensor_tensor (psum*1 + cv_rep) adds the bias and writes fp16
  staging (j-major [p, j, 384]).
* Stores are identity copies into device-order fp16 DRAM (128 x 12KB
  contiguous lines); the host inverts the layout permutation and upcasts.
  fp16 I/O halves both DMA streams; total rel err ~6e-4 vs fp32 reference.
"""

import numpy as np

H = 24
IN = 8
NCORES = 8
B_FULL = 1048576
F32 = None  # set lazily (mybir import) so numpy-only host code can be tested


def _np_softplus(x):
    x = np.asarray(x, np.float32)
    return np.logaddexp(np.float32(0.0), x).astype(np.float32)


def _np_sigmoid(x):
    x = np.asarray(x, np.float32)
    return (np.float32(1.0) / (np.float32(1.0) + np.exp(-x))).astype(np.float32)


def host_precompute(W, P, b_v, b_z, e, e_p, c_x, c_u, c_U, v0, X0, U0):
    """All (24,1)/(24,24) batch-constant math, in float32 mirroring the ref."""
    dt = np.float32(0.1)
    delta_t = np.float32(1.0)
    z_min, z_max = np.float32(0.001), np.float32(0.1)
    sp, sig = _np_softplus, _np_sigmoid

    W = np.asarray(W, np.float32)
    P = np.asarray(P, np.float32)
    b_v = np.asarray(b_v, np.float32).reshape(H, 1)
    b_z = np.asarray(b_z, np.float32).reshape(H, 1)
    v0 = np.asarray(v0, np.float32).reshape(H, 1)
    X0 = np.asarray(X0, np.float32).reshape(H, 1)
    U0 = np.asarray(U0, np.float32).reshape(H, 1)
    c_x = np.asarray(c_x, np.float32).reshape(H, 1)
    c_u = np.asarray(c_u, np.float32).reshape(H, 1)
    c_U = np.asarray(c_U, np.float32).reshape(H, 1)

    K = sp(np.float32(e).reshape(())) * sp(W)        # (H,H)
    P_z = sp(np.float32(e_p).reshape(())) * sp(P)    # (H,IN)

    r = sig(v0)                                      # (H,1)
    z_x = z_min + (z_max - z_min) * sig(c_x)
    X = z_x + (np.float32(1.0) - z_x) * X0 - delta_t * U0 * X0 * r
    z_u = z_min + (z_max - z_min) * sig(c_u)
    Ucap = np.float32(0.9) * sig(c_U)
    U = Ucap * z_u + (np.float32(1.0) - z_u) * U0 + delta_t * Ucap * (np.float32(1.0) - U0) * r
    U_c = np.clip(U, Ucap, np.float32(1.0))          # (H,1), batch-constant

    zpre = (K @ r + b_z).astype(np.float32)          # (H,1)
    u_vec = (U_c * X * r).astype(np.float32)         # (H,1)
    bias_v = (W @ u_vec + b_v).astype(np.float32)    # (H,1)

    w_v = (dt * P).T.astype(np.float32).copy()       # (IN,H)
    cv = (dt * bias_v + (np.float32(1.0) - dt) * v0).reshape(H).astype(np.float32)
    w_z = (-P_z).T.astype(np.float32).copy()         # (IN,H)
    cz = (-zpre).reshape(H).astype(np.float32)
    dtv0 = (dt * v0).reshape(H).astype(np.float32)
    return w_v, cv, w_z, cz, dtv0


def _block_diag(w, S):
    """w (IN,H) -> [128, S*H]; block c reads partitions {k*16+c} (k-major
    layout so the x shard loads as fully contiguous per-partition spans)."""
    out = np.zeros((128, S * H), np.float32)
    for c in range(S):
        for k in range(IN):
            out[k * S + c, H * c : H * c + H] = w[k]
    return out


def _pad_vec(v, S, PAIR):
    """v (H,) -> [1, PAIR*512]: tile(v, S) at cols 512*q..512*q+S*H per q."""
    out = np.zeros((1, PAIR * 512), np.float32)
    for q in range(PAIR):
        out[0, 512 * q : 512 * q + S * H] = np.tile(v, S)
    return out


def _qsched(total):
    """Split `total` (= B_c/2048) into per-supertile Q values: small head
    supertiles so the first matmul/store starts early, small tail so the
    final ship-out is short, 16s in the middle for 1536B store chunks."""
    if total < 16:
        return [total]
    if total < 48 or (total - 32) % 16:
        return [4, 12] + [16] * ((total - 16) // 16)
    # tiny head supertile (fast first matmul), tiny tail (short ship-out)
    return [2, 2, 12] + [16] * ((total - 32) // 16) + [8, 4, 2, 2]


def build_program(B_c, full_path, qsched=None):
    """Build the per-core Bass program.

    B_c: batches per core.  Chunk c = x columns [c*B_c/16, (c+1)*B_c/16);
    supertile T covers 128*qsched[T] consecutive batches of every chunk.
    full_path: include the sigmoid correction term (needed iff v0 != 0).
    """
    import concourse.bass as bass
    import concourse.bacc as bacc
    import concourse.tile as tile
    from concourse import mybir

    S = 16
    CHB = B_c // S           # batches (and x elems) per chunk
    qsched = qsched or _qsched(B_c // (S * 128))
    assert sum(128 * q for q in qsched) == CHB, (qsched, CHB)
    N = S * H                # matmul free dim = 384
    # G matmuls share one G-bank PSUM tile and one fused DVE pass (3D APs:
    # psum [p, q, N] <-> j-major staging [p, j, N]).  The output DMA is an
    # identity copy into device-order DRAM (host inverts the permutation),
    # so every store is 128 x JFc*768B fully-contiguous lines.
    G = 2 if full_path else 4
    f32 = mybir.dt.float32
    f16 = mybir.dt.float16

    nc = bacc.Bacc()
    x_in = nc.declare_dram_parameter("xs", [IN, B_c], f16, isOutput=False)
    wblk_in = nc.declare_dram_parameter("wblk", [128, N], f16, isOutput=False)
    cvec_in = nc.declare_dram_parameter("cvec", [1, G * 512], f32, isOutput=False)
    if full_path:
        wblkz_in = nc.declare_dram_parameter("wblkz", [128, N], f16, isOutput=False)
        czvec_in = nc.declare_dram_parameter("czvec", [1, G * 512], f32, isOutput=False)
        dvvec_in = nc.declare_dram_parameter("dvvec", [1, G * 512], f32, isOutput=False)
    out_ext = nc.declare_dram_parameter("out", [B_c * H], f16, isOutput=True)

    AT = mybir.AluOpType
    with tile.TileContext(nc) as tc:
        with (
            tc.tile_pool(name="singles", bufs=1) as singles,
            tc.tile_pool(name="op", bufs=4) as op,
            tc.tile_pool(name="ps", bufs=2, space="PSUM") as psp,
            tc.tile_pool(name="sp", bufs=4) as sbp,
        ):
            # x supertile 0 is triggered before everything else so the first
            # matmul's stationary lands ASAP (DMA ring has ~2us latency).
            SLICE0 = 128 * qsched[0]
            xt_first = singles.tile([128, SLICE0], f16)
            srcx0 = x_in[:, :].rearrange(
                "k (c w) -> k c w", c=S)[:, :, 0:SLICE0]
            nc.sync.dma_start(out=xt_first, in_=srcx0)

            wblk_sb = singles.tile([128, N], f16)
            nc.sync.dma_start(out=wblk_sb, in_=wblk_in[:, :])
            cv_rep = singles.tile([128, G * 512], f32)
            if full_path:
                wblkz_sb = singles.tile([128, N], f16)
                nc.sync.dma_start(out=wblkz_sb, in_=wblkz_in[:, :])
                cz_rep = singles.tile([128, G * 512], f32)
                dv_rep = singles.tile([128, G * 512], f32)

            def gv(t, g):
                """bank-padded [128, G*512] tile -> 3D [p, q<=g, N] view."""
                return t.rearrange("p (q b) -> p q b", q=G)[:, 0:g, 0:N]

            off = 0       # per-chunk element offset of this supertile's span
            flat = 0      # flat element offset into device-order output
            for T, QT in enumerate(qsched):
                SLICE = 128 * QT
                # ---- x load (f16, host-cast; sync HWDGE ring) ----
                # partition k*16+c <- x[k, c*CHB + off + w], w < SLICE
                if T == 0:
                    xt = xt_first          # triggered before wblk above
                else:
                    xt = singles.tile([128, SLICE], f16, tag=f"xt{T}")
                    srcx = x_in[:, :].rearrange(
                        "k (c w) -> k c w", c=S)[:, :, off : off + SLICE]
                    nc.sync.dma_start(out=xt[:, :], in_=srcx)
                if T == 0:
                    # one-time broadcasts (must be emitted before the first
                    # STT that reads them so Tile records the dependency)
                    nc.gpsimd.dma_start(
                        out=cv_rep, in_=cvec_in[:, :].to_broadcast([128, G * 512]))
                    if full_path:
                        nc.gpsimd.dma_start(
                            out=cz_rep,
                            in_=czvec_in[:, :].to_broadcast([128, G * 512]))
                        nc.gpsimd.dma_start(
                            out=dv_rep,
                            in_=dvvec_in[:, :].to_broadcast([128, G * 512]))

                # output flush plan within this supertile
                plan = [16] * (QT // 16) if QT > 16 else [QT]
                jbase = 0
                for JFc in plan:
                    # j-major staging: f = j*(S*H) + c*H + h
                    out_sb = op.tile([128, JFc * S * H], f16, tag="osb")
                    for j0 in range(0, JFc, G):
                        g = min(G, JFc - j0)
                        pt = psp.tile([128, G * 512], f32, tag="pt")
                        for q in range(g):
                            lhsT = xt.rearrange(
                                "p (m q) -> p m q", q=QT)[:, :, jbase + j0 + q]
                            nc.tensor.matmul(pt[:, 512 * q : 512 * q + N], lhsT,
                                             wblk_sb, start=True, stop=True)
                        p_v = gv(pt, g)
                        c_v = gv(cv_rep, g)
                        o_v = out_sb.rearrange(
                            "p (j b) -> p j b", b=S * H)[:, j0 : j0 + g, :]
                        if not full_path:
                            # out = ps + cv (fused copy+bias, one DVE pass)
                            nc.vector.scalar_tensor_tensor(
                                out=o_v, in0=p_v, scalar=1.0, in1=c_v,
                                op0=AT.mult, op1=AT.add,
                            )
                        else:
                            ptz = psp.tile([128, G * 512], f32, tag="ptz")
                            for q in range(g):
                                lhsT = xt.rearrange(
                                    "p (m q) -> p m q", q=QT)[:, :, jbase + j0 + q]
                                nc.tensor.matmul(ptz[:, 512 * q : 512 * q + N],
                                                 lhsT, wblkz_sb,
                                                 start=True, stop=True)
                            zb = sbp.tile([128, G * N], f32)
                            zb_v = zb.rearrange("p (q b) -> p q b", q=G)[:, 0:g, :]
                            # zb = psz + cz
                            nc.vector.scalar_tensor_tensor(
                                out=zb_v, in0=gv(ptz, g), scalar=1.0,
                                in1=gv(cz_rep, g), op0=AT.mult, op1=AT.add,
                            )
                            # s = sig(zb)
                            sg = sbp.tile([128, G * N], f32)
                            nc.scalar.activation(
                                out=sg, in_=zb,
                                func=mybir.ActivationFunctionType.Sigmoid,
                            )
                            sg_v = sg.rearrange("p (q b) -> p q b", q=G)[:, 0:g, :]
                            # t = sg * dtv0; t += cv; out = ps + t
                            tt = sbp.tile([128, G * N], f32)
                            tt_v = tt.rearrange("p (q b) -> p q b", q=G)[:, 0:g, :]
                            nc.vector.tensor_tensor(
                                out=tt_v, in0=sg_v, in1=gv(dv_rep, g), op=AT.mult,
                            )
                            nc.vector.scalar_tensor_tensor(
                                out=tt_v, in0=tt_v, scalar=1.0, in1=c_v,
                                op0=AT.mult, op1=AT.add,
                            )
                            nc.vector.scalar_tensor_tensor(
                                out=o_v, in0=gv(pt, g), scalar=1.0, in1=tt_v,
                                op0=AT.mult, op1=AT.add,
                            )

                    # ---- out DMA: identity copy into device-order DRAM ----
                    sz = 128 * JFc * S * H
                    dst_o = out_ext[flat : flat + sz].rearrange(
                        "(m f) -> m f", m=128)
                    nc.scalar.dma_start(out=dst_o, in_=out_sb[:, :])
                    flat += sz
                    jbase += JFc
                off += SLICE
    nc.compile()  # bacc legalization: wait-splitting, event sems, table loads
    return nc


def unshard_core(dev_flat, qsched, B_c):
    """Invert the device-order output layout -> (B_c, H) float32."""
    S = 16
    CHB = B_c // S
    out_core = np.empty((S, CHB, H), np.float32)
    flat = 0
    off = 0
    for QT in qsched:
        plan = [16] * (QT // 16) if QT > 16 else [QT]
        jbase = 0
        dst = out_core[:, off : off + 128 * QT, :]    # view (S, 128*QT, H)
        for JFc in plan:
            sz = 128 * JFc * S * H
            piece = np.asarray(dev_flat[flat : flat + sz]).reshape(
                128, JFc, S, H).astype(np.float32)
            idx = (np.arange(128)[:, None] * QT + jbase
                   + np.arange(JFc)[None, :]).ravel()
            dst[:, idx, :] = piece.transpose(2, 0, 1, 3).reshape(S, 128 * JFc, H)
            flat += sz
            jbase += JFc
        off += 128 * QT
    return out_core.reshape(B_c, H)


def _run(nc, in_maps, core_ids, trace=False):
    from concourse.bass_utils import run_bass_kernel_spmd
    return run_bass_kernel_spmd(nc, in_maps, core_ids, trace=trace)


def kernel(x, W, P, b_v, b_z, e, e_p, c_x, c_u, c_U, v0, X0, U0,
           _trace=False, _qs=None):
    x = np.ascontiguousarray(np.asarray(x, np.float32))
    assert x.shape == (IN, B_FULL), x.shape
    w_v, cv, w_z, cz, dtv0 = host_precompute(
        W, P, b_v, b_z, e, e_p, c_x, c_u, c_U, v0, X0, U0)
    full_path = bool(np.any(dtv0 != 0))

    S = 16
    G = 2 if full_path else 4
    B_c = B_FULL // NCORES
    qsched = _qs or _qsched(B_c // (S * 128))
    nc = build_program(B_c, full_path, qsched=qsched)

    wblk = _block_diag(w_v, S).astype(np.float16)
    base = {"wblk": wblk, "cvec": _pad_vec(cv, S, G)}
    if full_path:
        base["wblkz"] = _block_diag(w_z, S).astype(np.float16)
        base["czvec"] = _pad_vec(cz, S, G)
        base["dvvec"] = _pad_vec(dtv0, S, G)

    core_ids = list(range(NCORES))
    in_maps = []
    for c in core_ids:
        m = dict(base)
        m["xs"] = np.ascontiguousarray(
            x[:, c * B_c : (c + 1) * B_c]).astype(np.float16)
        in_maps.append(m)

    res = _run(nc, in_maps, core_ids, trace=_trace)
    out = np.concatenate(
        [unshard_core(res.results[i]["out"], qsched, B_c)
         for i in range(NCORES)], axis=0)
    if _trace:
        kernel.last_exec_time_ns = res.exec_time_ns
        kernel.last_results = res
    return out



# revision 38
# speedup vs baseline: 1.3295x; 1.0011x over previous
"""Trainium2 Bass kernel for nn_CB_RNN_tiedcell (H=24, IN=8, B=1048576).

Math
----
reference(x, W, P, ...) computes, per batch column b:
    z_t = dt*sig(K@r + P_z@x_b + b_z)      (K, P_z, r, biases batch-constant)
    v   = (1-z_t)*v0 + dt*(W@(U*X*r) + P@x_b + b_v)
All (24,1) state math (r, X, U, Ucap, clamp, K@r, W@u) is batch-constant and
precomputed on the host.  With s = sig(-(P_z@x_b + zpre)) = 1 - sig(+...):
    v[:,b] = dt*P@x_b + cv + dtv0 * s[:,b]
where cv = dt*(W@u + b_v) + (1-dt)*v0 and dtv0 = dt*v0.  When v0 == 0 (the
shipped inputs) the sigmoid path vanishes; the program is built without it
(full_path=False) and a general program is built when v0 != 0.

Kernel design (pure data parallel, 8 cores, B/8 = 131072 batches each)
----------------------------------------------------------------------
* Block-diagonal stationary trick: one fp16 matmul per 2048 batches.  The
  PE stationary is a [128, 128] tile of x holding 16 independent 8-row
  sub-chunks (chunk c of the shard on partitions {k*16+c}); the moving
  operand is a constant block-diagonal weight matrix [128, 16*24].  One
  matmul yields batch-major [128, 384] PSUM = 16 chunks x 128 batches.
* x is host-cast to fp16 and laid out so every per-partition DMA span is
  fully contiguous; supertile sizes ramp 4,12,16,...,8,4,4 so the first
  matmul and first store start early and the final ship-out tail is short.
* 4 matmuls share one 4-bank PSUM tile; a single fused DVE
  scalar_tensor_tensor (psum*1 + cv_rep) adds the bias and writes fp16
  staging (j-major [p, j, 384]).
* Stores are identity copies into device-order fp16 DRAM (128 x 12KB
  contiguous lines); the host inverts the layout permutation and upcasts.
  fp16 I/O halves both DMA streams; total rel err ~6e-4 vs fp32 reference.
"""

import numpy as np

H = 24
IN = 8
NCORES = 8
B_FULL = 1048576
F32 = None  # set lazily (mybir import) so numpy-only host code can be tested


def _np_softplus(x):
    x = np.asarray(x, np.float32)
    return np.logaddexp(np.float32(0.0), x).astype(np.float32)


def _np_sigmoid(x):
    x = np.asarray(x, np.float32)
    return (np.float32(1.0) / (np.float32(1.0) + np.exp(-x))).astype(np.float32)


def host_precompute(W, P, b_v, b_z, e, e_p, c_x, c_u, c_U, v0, X0, U0):
    """All (24,1)/(24,24) batch-constant math, in float32 mirroring the ref."""
    dt = np.float32(0.1)
    delta_t = np.float32(1.0)
    z_min, z_max = np.float32(0.001), np.float32(0.1)
    sp, sig = _np_softplus, _np_sigmoid

    W = np.asarray(W, np.float32)
    P = np.asarray(P, np.float32)
    b_v = np.asarray(b_v, np.float32).reshape(H, 1)
    b_z = np.asarray(b_z, np.float32).reshape(H, 1)
    v0 = np.asarray(v0, np.float32).reshape(H, 1)
    X0 = np.asarray(X0, np.float32).reshape(H, 1)
    U0 = np.asarray(U0, np.float32).reshape(H, 1)
    c_x = np.asarray(c_x, np.float32).reshape(H, 1)
    c_u = np.asarray(c_u, np.float32).reshape(H, 1)
    c_U = np.asarray(c_U, np.float32).reshape(H, 1)

    K = sp(np.float32(e).reshape(())) * sp(W)        # (H,H)
    P_z = sp(np.float32(e_p).reshape(())) * sp(P)    # (H,IN)

    r = sig(v0)                                      # (H,1)
    z_x = z_min + (z_max - z_min) * sig(c_x)
    X = z_x + (np.float32(1.0) - z_x) * X0 - delta_t * U0 * X0 * r
    z_u = z_min + (z_max - z_min) * sig(c_u)
    Ucap = np.float32(0.9) * sig(c_U)
    U = Ucap * z_u + (np.float32(1.0) - z_u) * U0 + delta_t * Ucap * (np.float32(1.0) - U0) * r
    U_c = np.clip(U, Ucap, np.float32(1.0))          # (H,1), batch-constant

    zpre = (K @ r + b_z).astype(np.float32)          # (H,1)
    u_vec = (U_c * X * r).astype(np.float32)         # (H,1)
    bias_v = (W @ u_vec + b_v).astype(np.float32)    # (H,1)

    w_v = (dt * P).T.astype(np.float32).copy()       # (IN,H)
    cv = (dt * bias_v + (np.float32(1.0) - dt) * v0).reshape(H).astype(np.float32)
    w_z = (-P_z).T.astype(np.float32).copy()         # (IN,H)
    cz = (-zpre).reshape(H).astype(np.float32)
    dtv0 = (dt * v0).reshape(H).astype(np.float32)
    return w_v, cv, w_z, cz, dtv0


def _block_diag(w, S):
    """w (IN,H) -> [128, S*H]; block c reads partitions {k*16+c} (k-major
    layout so the x shard loads as fully contiguous per-partition spans)."""
    out = np.zeros((128, S * H), np.float32)
    for c in range(S):
        for k in range(IN):
            out[k * S + c, H * c : H * c + H] = w[k]
    return out


def _pad_vec(v, S, PAIR):
    """v (H,) -> [1, PAIR*512]: tile(v, S) at cols 512*q..512*q+S*H per q."""
    out = np.zeros((1, PAIR * 512), np.float32)
    for q in range(PAIR):
        out[0, 512 * q : 512 * q + S * H] = np.tile(v, S)
    return out


def _qsched(total):
    """Split `total` (= B_c/2048) into per-supertile Q values: small head
    supertiles so the first matmul/store starts early, small tail so the
    final ship-out is short, 16s in the middle for 1536B store chunks."""
    if total < 16:
        return [total]
    if total < 48 or (total - 32) % 16:
        return [4, 12] + [16] * ((total - 16) // 16)
    # tiny head supertile (fast first matmul), tiny tail (short ship-out)
    return [2, 2, 12] + [16] * ((total - 32) // 16) + [8, 4, 2, 2]


def build_program(B_c, full_path, qsched=None):
    """Build the per-core Bass program.

    B_c: batches per core.  Chunk c = x columns [c*B_c/16, (c+1)*B_c/16);
    supertile T covers 128*qsched[T] consecutive batches of every chunk.
    full_path: include the sigmoid correction term (needed iff v0 != 0).
    """
    import concourse.bass as bass
    import concourse.bacc as bacc
    import concourse.tile as tile
    from concourse import mybir

    S = 16
    CHB = B_c // S           # batches (and x elems) per chunk
    qsched = qsched or _qsched(B_c // (S * 128))
    assert sum(128 * q for q in qsched) == CHB, (qsched, CHB)
    N = S * H                # matmul free dim = 384
    # G matmuls share one G-bank PSUM tile and one fused DVE pass (3D APs:
    # psum [p, q, N] <-> j-major staging [p, j, N]).  The output DMA is an
    # identity copy into device-order DRAM (host inverts the permutation),
    # so every store is 128 x JFc*768B fully-contiguous lines.
    G = 2 if full_path else 4
    f32 = mybir.dt.float32
    f16 = mybir.dt.float16

    nc = bacc.Bacc()
    x_in = nc.declare_dram_parameter("xs", [IN, B_c], f16, isOutput=False)
    wblk_in = nc.declare_dram_parameter("wblk", [128, N], f16, isOutput=False)
    cvrep_in = nc.declare_dram_parameter("cvrep", [128, G * 512], f32,
                                         isOutput=False)
    if full_path:
        cvec_in = nc.declare_dram_parameter("cvec", [1, G * 512], f32, isOutput=False)
        wblkz_in = nc.declare_dram_parameter("wblkz", [128, N], f16, isOutput=False)
        czvec_in = nc.declare_dram_parameter("czvec", [1, G * 512], f32, isOutput=False)
        dvvec_in = nc.declare_dram_parameter("dvvec", [1, G * 512], f32, isOutput=False)
    out_ext = nc.declare_dram_parameter("out", [B_c * H], f16, isOutput=True)

    AT = mybir.AluOpType
    with tile.TileContext(nc) as tc:
        with (
            tc.tile_pool(name="singles", bufs=1) as singles,
            tc.tile_pool(name="op", bufs=4) as op,
            tc.tile_pool(name="ps", bufs=2, space="PSUM") as psp,
            tc.tile_pool(name="sp", bufs=4) as sbp,
        ):
            # x supertile 0 is triggered before everything else so the first
            # matmul's stationary lands ASAP (DMA ring has ~2us latency).
            SLICE0 = 128 * qsched[0]
            xt_first = singles.tile([128, SLICE0], f16)
            srcx0 = x_in[:, :].rearrange(
                "k (c w) -> k c w", c=S)[:, :, 0:SLICE0]
            nc.sync.dma_start(out=xt_first, in_=srcx0)

            wblk_sb = singles.tile([128, N], f16)
            nc.sync.dma_start(out=wblk_sb, in_=wblk_in[:, :])
            # cv_rep arrives host-materialized over the scalar HWDGE ring
            # (idle until stores begin) so the 1 MiB transfer never contends
            # with x supertiles on the sync ring — the old software-DGE
            # broadcast delayed supertile 0 by ~4.5us.
            cv_rep = singles.tile([128, G * 512], f32)
            nc.scalar.dma_start(out=cv_rep, in_=cvrep_in[:, :])
            if full_path:
                wblkz_sb = singles.tile([128, N], f16)
                nc.sync.dma_start(out=wblkz_sb, in_=wblkz_in[:, :])
                cz_rep = singles.tile([128, G * 512], f32)
                dv_rep = singles.tile([128, G * 512], f32)

            def gv(t, g):
                """bank-padded [128, G*512] tile -> 3D [p, q<=g, N] view."""
                return t.rearrange("p (q b) -> p q b", q=G)[:, 0:g, 0:N]

            off = 0       # per-chunk element offset of this supertile's span
            flat = 0      # flat element offset into device-order output
            for T, QT in enumerate(qsched):
                SLICE = 128 * QT
                # ---- x load (f16, host-cast; sync HWDGE ring) ----
                # partition k*16+c <- x[k, c*CHB + off + w], w < SLICE
                if T == 0:
                    xt = xt_first          # triggered before wblk above
                else:
                    xt = singles.tile([128, SLICE], f16, tag=f"xt{T}")
                    srcx = x_in[:, :].rearrange(
                        "k (c w) -> k c w", c=S)[:, :, off : off + SLICE]
                    nc.sync.dma_start(out=xt[:, :], in_=srcx)
                if T == 0:
                    if full_path:
                        nc.gpsimd.dma_start(
                            out=cz_rep,
                            in_=czvec_in[:, :].to_broadcast([128, G * 512]))
                        nc.gpsimd.dma_start(
                            out=dv_rep,
                            in_=dvvec_in[:, :].to_broadcast([128, G * 512]))

                # output flush plan within this supertile
                plan = [16] * (QT // 16) if QT > 16 else [QT]
                jbase = 0
                for JFc in plan:
                    # j-major staging: f = j*(S*H) + c*H + h
                    out_sb = op.tile([128, JFc * S * H], f16, tag="osb")
                    for j0 in range(0, JFc, G):
                        g = min(G, JFc - j0)
                        pt = psp.tile([128, G * 512], f32, tag="pt")
                        for q in range(g):
                            lhsT = xt.rearrange(
                                "p (m q) -> p m q", q=QT)[:, :, jbase + j0 + q]
                            nc.tensor.matmul(pt[:, 512 * q : 512 * q + N], lhsT,
                                             wblk_sb, start=True, stop=True)
                        p_v = gv(pt, g)
                        c_v = gv(cv_rep, g)
                        o_v = out_sb.rearrange(
                            "p (j b) -> p j b", b=S * H)[:, j0 : j0 + g, :]
                        if not full_path:
                            # out = ps + cv (fused copy+bias, one DVE pass)
                            nc.vector.scalar_tensor_tensor(
                                out=o_v, in0=p_v, scalar=1.0, in1=c_v,
                                op0=AT.mult, op1=AT.add,
                            )
                        else:
                            ptz = psp.tile([128, G * 512], f32, tag="ptz")
                            for q in range(g):
                                lhsT = xt.rearrange(
                                    "p (m q) -> p m q", q=QT)[:, :, jbase + j0 + q]
                                nc.tensor.matmul(ptz[:, 512 * q : 512 * q + N],
                                                 lhsT, wblkz_sb,
                                                 start=True, stop=True)
                            zb = sbp.tile([128, G * N], f32)
                            zb_v = zb.rearrange("p (q b) -> p q b", q=G)[:, 0:g, :]
                            # zb = psz + cz
                            nc.vector.scalar_tensor_tensor(
                                out=zb_v, in0=gv(ptz, g), scalar=1.0,
                                in1=gv(cz_rep, g), op0=AT.mult, op1=AT.add,
                            )
                            # s = sig(zb)
                            sg = sbp.tile([128, G * N], f32)
                            nc.scalar.activation(
                                out=sg, in_=zb,
                                func=mybir.ActivationFunctionType.Sigmoid,
                            )
                            sg_v = sg.rearrange("p (q b) -> p q b", q=G)[:, 0:g, :]
                            # t = sg * dtv0; t += cv; out = ps + t
                            tt = sbp.tile([128, G * N], f32)
                            tt_v = tt.rearrange("p (q b) -> p q b", q=G)[:, 0:g, :]
                            nc.vector.tensor_tensor(
                                out=tt_v, in0=sg_v, in1=gv(dv_rep, g), op=AT.mult,
                            )
                            nc.vector.scalar_tensor_tensor(
                                out=tt_v, in0=tt_v, scalar=1.0, in1=c_v,
                                op0=AT.mult, op1=AT.add,
                            )
                            nc.vector.scalar_tensor_tensor(
                                out=o_v, in0=gv(pt, g), scalar=1.0, in1=tt_v,
                                op0=AT.mult, op1=AT.add,
                            )

                    # ---- out DMA: identity copy into device-order DRAM ----
                    sz = 128 * JFc * S * H
                    dst_o = out_ext[flat : flat + sz].rearrange(
                        "(m f) -> m f", m=128)
                    nc.scalar.dma_start(out=dst_o, in_=out_sb[:, :])
                    flat += sz
                    jbase += JFc
                off += SLICE
    nc.compile()  # bacc legalization: wait-splitting, event sems, table loads
    return nc


def unshard_core(dev_flat, qsched, B_c):
    """Invert the device-order output layout -> (B_c, H) float32."""
    S = 16
    CHB = B_c // S
    out_core = np.empty((S, CHB, H), np.float32)
    flat = 0
    off = 0
    for QT in qsched:
        plan = [16] * (QT // 16) if QT > 16 else [QT]
        jbase = 0
        dst = out_core[:, off : off + 128 * QT, :]    # view (S, 128*QT, H)
        for JFc in plan:
            sz = 128 * JFc * S * H
            piece = np.asarray(dev_flat[flat : flat + sz]).reshape(
                128, JFc, S, H).astype(np.float32)
            idx = (np.arange(128)[:, None] * QT + jbase
                   + np.arange(JFc)[None, :]).ravel()
            dst[:, idx, :] = piece.transpose(2, 0, 1, 3).reshape(S, 128 * JFc, H)
            flat += sz
            jbase += JFc
        off += 128 * QT
    return out_core.reshape(B_c, H)


def _run(nc, in_maps, core_ids, trace=False):
    from concourse.bass_utils import run_bass_kernel_spmd
    return run_bass_kernel_spmd(nc, in_maps, core_ids, trace=trace)


def kernel(x, W, P, b_v, b_z, e, e_p, c_x, c_u, c_U, v0, X0, U0,
           _trace=False, _qs=None):
    x = np.ascontiguousarray(np.asarray(x, np.float32))
    assert x.shape == (IN, B_FULL), x.shape
    w_v, cv, w_z, cz, dtv0 = host_precompute(
        W, P, b_v, b_z, e, e_p, c_x, c_u, c_U, v0, X0, U0)
    full_path = bool(np.any(dtv0 != 0))

    S = 16
    G = 2 if full_path else 4
    B_c = B_FULL // NCORES
    qsched = _qs or _qsched(B_c // (S * 128))
    nc = build_program(B_c, full_path, qsched=qsched)

    wblk = _block_diag(w_v, S).astype(np.float16)
    cvp = _pad_vec(cv, S, G)
    base = {"wblk": wblk,
            "cvrep": np.ascontiguousarray(
                np.broadcast_to(cvp, (128, G * 512))).astype(np.float32)}
    if full_path:
        base["cvec"] = cvp
        base["wblkz"] = _block_diag(w_z, S).astype(np.float16)
        base["czvec"] = _pad_vec(cz, S, G)
        base["dvvec"] = _pad_vec(dtv0, S, G)

    core_ids = list(range(NCORES))
    in_maps = []
    for c in core_ids:
        m = dict(base)
        m["xs"] = np.ascontiguousarray(
            x[:, c * B_c : (c + 1) * B_c]).astype(np.float16)
        in_maps.append(m)

    res = _run(nc, in_maps, core_ids, trace=_trace)
    out = np.concatenate(
        [unshard_core(res.results[i]["out"], qsched, B_c)
         for i in range(NCORES)], axis=0)
    if _trace:
        kernel.last_exec_time_ns = res.exec_time_ns
        kernel.last_results = res
    return out

